# revision 1
# baseline (speedup 1.0000x reference)
"""Trainium2 Bass kernel for nn_Brep_Gcn (GCN message passing).

Math (reference):
    x  = relu(sum_ch conv1d(feature))            # conv folds to a banded matmul
    h  = relu(A @ (x W1) + b1) = relu((A @ x) W1 + b1)
    y  = A @ (h W2) + b2       = (A @ h?? no) -> A @ P where P = h W2

Reordered for memory efficiency:
    S1 = A @ x          (sparse gather + segment-sum, 83-wide rows)
    h  = relu(S1 W1 + b1)
    P  = h W2
    y  = A @ P + b2     (sparse gather + segment-sum, 25-wide rows)

Distribution: nodes row-sharded across 8 cores; edges partitioned by
destination owner; x and P replicated via AllGather; weights replicated.

Sparse segment-sum on device: edges sorted by (dest-window, src-chunk),
padded to 128-edge blocks.  Per block: dma_gather the 128 source rows,
build a one-hot selector Sel[e, d] = val[e] * (slot[e] == d) with one fused
tensor_scalar on DVE, and matmul on the PE accumulating into PSUM per
(window, chunk) segment.
"""

import math
import os
import sys
from dataclasses import dataclass

import numpy as np

sys.path.insert(0, "/opt/trn_rl_repo")

import concourse.bass as bass
import concourse.tile as tile
from concourse import bacc
from concourse import mybir
from concourse.bass_utils import run_bass_kernel_spmd
from concourse.masks import make_identity

F32 = mybir.dt.float32
I16 = mybir.dt.int16
I32 = mybir.dt.int32
AF = mybir.ActivationFunctionType
OP = mybir.AluOpType


@dataclass
class Cfg:
    N: int = 100000
    E: int = 3200000
    D_IN: int = 83
    D_HID: int = 1024
    NCLS: int = 25
    NCORES: int = 8
    NCHUNK: int = 4          # source-index chunks (int16 gather indices)
    XPAD: int = 128          # padded x row, f32 (512 B, stride mult of 256 B)
    PPAD: int = 64           # padded P row, f32 (256 B)
    GBLK: int = 8            # max 128-edge blocks per dma_gather call
                             # (HW SWDGE ring limit: 1024 idxs per call)
    IDXG: int = 8            # gather calls per idx-staging DMA

    @property
    def NSH(self):
        return self.N // self.NCORES

    @property
    def CHUNK(self):
        return self.N // self.NCHUNK

    @property
    def NW(self):            # dest windows (of 128) per core
        return (self.NSH + 127) // 128

    @property
    def NJ(self):            # hidden dim in 128-blocks
        return self.D_HID // 128


# ----------------------------------------------------------------------------
# Host-side preprocessing
# ----------------------------------------------------------------------------

def _wrap_idx16(idx: np.ndarray) -> np.ndarray:
    """dma_gather index layout: idx i at [i % 16, i // 16], tiled to 128
    partitions (replicated for the 8 Q7 cores)."""
    assert idx.size % 16 == 0
    a = idx.reshape(-1, 16).T.astype(np.int16)       # [16, n/16]
    return np.tile(a, (8, 1))                        # [128, n/16]


def build_host(cfg: Cfg, inputs: dict) -> tuple[list[dict], dict]:
    """Returns (per-core input maps, shared structure metadata)."""
    N, E = cfg.N, cfg.E
    NSH, NW, NCH, CH = cfg.NSH, cfg.NW, cfg.NCHUNK, cfg.CHUNK

    feature = np.asarray(inputs["feature"], np.float32)
    conv_w = np.asarray(inputs["conv_w"], np.float32)
    conv_b = np.asarray(inputs["conv_b"], np.float32)
    W1 = np.asarray(inputs["W1"], np.float32)
    b1 = np.asarray(inputs["b1"], np.float32)
    W2 = np.asarray(inputs["W2"], np.float32)
    b2 = np.asarray(inputs["b2"], np.float32)
    val = np.asarray(inputs["adj_val"], np.float32)
    row = np.asarray(inputs["edge_row"], np.int64)
    col = np.asarray(inputs["edge_col"], np.int64)

    # conv1d(1->4, k=5, pad 2) summed over channels == banded matmul.
    ws = conv_w.sum(axis=0).ravel()                  # [5]
    b0 = float(conv_b.sum())
    C = np.zeros((cfg.D_IN, cfg.XPAD), np.float32)
    for i in range(cfg.D_IN):
        for k in range(5):
            j = i - (k - 2)                          # out[:, j] += ws[k] * in[:, j + k - 2]
            if 0 <= j < cfg.D_IN:
                C[i, j] = ws[k]

    # ---- edge partitioning: by dest core, then (dest-window, src-chunk) ----
    core_of = row // NSH
    per_core = []
    cnt = np.zeros((cfg.NCORES, NW, NCH), np.int64)
    for k in range(cfg.NCORES):
        m = core_of == k
        r, c_, v = row[m] - k * NSH, col[m], val[m]
        w = r >> 7
        ch = c_ // CH
        order = np.lexsort((c_, ch, w))
        r, c_, v, w, ch = r[order], c_[order], v[order], w[order], ch[order]
        # counts per (w, chunk)
        key = w * NCH + ch
        cnt[k] = np.bincount(key, minlength=NW * NCH).reshape(NW, NCH)
        per_core.append((r, c_, v, key))

    # uniform block counts across cores
    M = np.maximum(1, np.ceil(cnt.max(axis=0) / 128).astype(np.int64))  # [NW, NCH]

    # block metadata, chunk-major (same for every core)
    blocks = []      # (w, chunk, seg_first, seg_last)
    calls = []       # (chunk, blk_start, nblk, idx_off16)
    nblk_total = int(M.sum())
    for ch in range(NCH):
        cblks = []
        for w in range(NW):
            for m in range(int(M[w, ch])):
                cblks.append((w, ch, m == 0, m == int(M[w, ch]) - 1))
        s = 0
        while s < len(cblks):
            n = min(cfg.GBLK, len(cblks) - s)
            calls.append([ch, len(blocks) + s, n, 0])
            s += n
        blocks.extend(cblks)
    assert len(blocks) == nblk_total
    # idx free-dim offsets (int16 units /16) per call
    off = 0
    for call in calls:
        call[3] = off
        off += call[2] * 128 // 16
    tot16 = off

    # ---- per-core padded edge arrays in block order ----
    in_maps = []
    for k in range(cfg.NCORES):
        r, c_, v, key = per_core[k]
        # segment boundaries in the sorted arrays
        seg_of = {}
        pos = np.searchsorted(key, np.arange(NW * NCH + 1), side="left")
        idx_pad = np.zeros(nblk_total * 128, np.int16)
        slot_pad = np.zeros(nblk_total * 128, np.float32)
        val_pad = np.zeros(nblk_total * 128, np.float32)
        bi = 0
        for ch in range(NCH):
            for w in range(NW):
                a, b = pos[w * NCH + ch], pos[w * NCH + ch + 1]
                n = b - a
                mb = int(M[w, ch])
                dst = bi * 128
                idx_pad[dst:dst + n] = (c_[a:b] % CH).astype(np.int16)
                slot_pad[dst:dst + n] = (r[a:b] - (w << 7)).astype(np.float32)
                val_pad[dst:dst + n] = v[a:b]
                bi += mb
        assert bi == nblk_total
        # idx in per-call wrapped layout, concatenated on the free dim
        idx_arr = np.zeros((128, tot16), np.int16)
        for ch, bs, nb, o16 in calls:
            seg = idx_pad[bs * 128:(bs + nb) * 128]
            idx_arr[:, o16:o16 + nb * 128 // 16] = _wrap_idx16(seg)
        slot_arr = slot_pad.reshape(nblk_total, 128).T.copy()
        val_arr = val_pad.reshape(nblk_total, 128).T.copy()

        b1c = b1.reshape(cfg.NJ, 128).T.copy()                    # [128, NJ]
        W2p = np.zeros((cfg.D_HID, cfg.PPAD), np.float32)
        W2p[:, :cfg.NCLS] = W2
        b2t = np.zeros((128, cfg.PPAD), np.float32)
        b2t[:, :cfg.NCLS] = b2[None, :]

        in_maps.append({
            "feat_sh": feature[k * NSH:(k + 1) * NSH],
            "Cmat": C,
            "W1": W1,
            "b1c": b1c,
            "W2p": W2p,
            "b2t": b2t,
            "idx_dr": idx_arr,
            "slot_dr": slot_arr,
            "val_dr": val_arr,
        })

    meta = {"blocks": blocks, "calls": calls, "nblk": nblk_total,
            "tot16": tot16, "b0": b0}
    return in_maps, meta


# ----------------------------------------------------------------------------
# Bass program (identical for every core; per-core data comes via inputs)
# ----------------------------------------------------------------------------

def build_program(cfg: Cfg, meta: dict, phases: str = "ABCD", max_calls: int = 10**9) -> bass.Bass:
    NSH, NW, NCH, CH = cfg.NSH, cfg.NW, cfg.NCHUNK, cfg.CHUNK
    NJ, XP, PP = cfg.NJ, cfg.XPAD, cfg.PPAD
    blocks, calls = meta["blocks"], meta["calls"]
    nblk, tot16 = meta["nblk"], meta["tot16"]
    groups = [list(range(cfg.NCORES))]

    nc = bacc.Bacc("TRN2", target_bir_lowering=False, debug=False,
                   num_devices=cfg.NCORES)

    feat_sh = nc.declare_dram_parameter("feat_sh", [NSH, cfg.D_IN], F32, isOutput=False)
    Cmat = nc.declare_dram_parameter("Cmat", [cfg.D_IN, XP], F32, isOutput=False)
    W1 = nc.declare_dram_parameter("W1", [cfg.D_IN, cfg.D_HID], F32, isOutput=False)
    b1c = nc.declare_dram_parameter("b1c", [128, NJ], F32, isOutput=False)
    W2p = nc.declare_dram_parameter("W2p", [cfg.D_HID, PP], F32, isOutput=False)
    b2t = nc.declare_dram_parameter("b2t", [128, PP], F32, isOutput=False)
    idx_dr = nc.declare_dram_parameter("idx_dr", [128, tot16], I16, isOutput=False)
    slot_dr = nc.declare_dram_parameter("slot_dr", [128, nblk], F32, isOutput=False)
    val_dr = nc.declare_dram_parameter("val_dr", [128, nblk], F32, isOutput=False)
    logits = nc.declare_dram_parameter("logits", [NSH, cfg.NCLS], F32, isOutput=True)

    if "J" in phases:
        x_full = nc.declare_dram_parameter("x_full_in", [cfg.N, XP], F32,
                                           isOutput=False)
        phases = phases.replace("J", "B")
    else:
        x_full = nc.dram_tensor("x_full", [cfg.N, XP], F32, addr_space="Shared")
    x_sh = nc.dram_tensor("x_sh", [NSH, XP], F32)
    p_sh = nc.dram_tensor("p_sh", [NSH, PP], F32)
    p_full = nc.dram_tensor("p_full", [cfg.N, PP], F32, addr_space="Shared")

    with tile.TileContext(nc) as tc:
        with (
            tc.tile_pool(name="singles", bufs=1) as singles,
            tc.tile_pool(name="work", bufs=3) as work,
            tc.tile_pool(name="sel", bufs=6) as selp,
            tc.tile_pool(name="gath", bufs=4) as gathp,
            tc.tile_pool(name="ht", bufs=18) as htp,
            tc.tile_pool(name="ps4", bufs=4, space="PSUM") as ps4,
            tc.tile_pool(name="psg", bufs=2, space="PSUM") as psg,
            tc.tile_pool(name="psp", bufs=2, space="PSUM") as psp,
        ):
            # ---------------- constants ----------------
            C_sb = singles.tile([cfg.D_IN, XP], F32)
            nc.sync.dma_start(out=C_sb[:], in_=Cmat[:])
            W1_sb = singles.tile([cfg.D_IN, cfg.D_HID], F32)
            nc.sync.dma_start(out=W1_sb[:], in_=W1[:])
            b1_sb = singles.tile([128, NJ], F32)
            nc.sync.dma_start(out=b1_sb[:], in_=b1c[:])
            W2_sb = singles.tile([128, NJ, PP], F32)
            nc.sync.dma_start(out=W2_sb[:], in_=W2p.rearrange("(j p) q -> p j q", p=128))
            b2_sb = singles.tile([128, PP], F32)
            nc.sync.dma_start(out=b2_sb[:], in_=b2t[:])
            slot_sb = singles.tile([128, nblk], F32)
            nc.sync.dma_start(out=slot_sb[:], in_=slot_dr[:])
            val_sb = singles.tile([128, nblk], F32)
            nc.sync.dma_start(out=val_sb[:], in_=val_dr[:])

            b0_sb = singles.tile([128, 1], F32)
            nc.vector.memset(b0_sb[:], meta["b0"])
            ident = singles.tile([128, 128], F32)
            make_identity(nc, ident[:])
            iota_i = singles.tile([128, 128], I32)
            nc.gpsimd.iota(iota_i[:], pattern=[[1, 128]], base=0, channel_multiplier=0)
            iota_f = singles.tile([128, 128], F32)
            nc.vector.tensor_copy(out=iota_f[:], in_=iota_i[:])

            S1T = singles.tile([cfg.D_IN, NSH], F32)
            nc.vector.memset(S1T[:], 0.0)
            logit_sb = singles.tile([128, NW, PP], F32)
            # logits init = b2 broadcast (0-step middle dim)
            b2_ap = b2_sb[:]
            b2_bc = bass.AP(tensor=b2_ap.tensor, offset=b2_ap.offset,
                            ap=[b2_ap.ap[0], [0, NW], b2_ap.ap[1]])
            nc.vector.tensor_copy(out=logit_sb[:], in_=b2_bc)

            # ---------------- phase A: conv shard + AllGather x ----------------
            for t in (range(NW) if "A" in phases else []):
                rows = min(128, NSH - t * 128)
                ft = work.tile([128, cfg.D_IN], F32, tag="ft")
                nc.sync.dma_start(out=ft[:rows], in_=feat_sh[t * 128:t * 128 + rows])
                ps_t = ps4.tile([128, 128], F32, tag="ps")
                nc.tensor.transpose(out=ps_t[:cfg.D_IN, :rows], in_=ft[:rows], identity=ident[:rows, :rows])
                ftT = work.tile([cfg.D_IN, 128], F32, tag="ftT")
                nc.scalar.activation(out=ftT[:, :rows], in_=ps_t[:cfg.D_IN, :rows], func=AF.Copy)
                ps_x = ps4.tile([128, XP], F32, tag="ps")
                nc.tensor.matmul(out=ps_x[:rows], lhsT=ftT[:, :rows], rhs=C_sb[:],
                                 start=True, stop=True)
                xt = work.tile([128, XP], F32, tag="xt")
                nc.scalar.activation(out=xt[:rows], in_=ps_x[:rows], func=AF.Relu,
                                     bias=b0_sb[:rows])
                nc.sync.dma_start(out=x_sh[t * 128:t * 128 + rows], in_=xt[:rows])

            if "A" in phases:
                nc.gpsimd.collective_compute(
                    "AllGather", OP.bypass, replica_groups=groups,
                    ins=[x_sh[:]], outs=[x_full[:]])
                tc.strict_bb_all_engine_barrier()

            if "B" not in phases:
                nc.sync.dma_start(out=logits[:],
                                  in_=x_full[:cfg.NSH, :cfg.NCLS])
            # ---------------- phase B: L1 SpMM  S1T = (A @ x).T ----------------
            ps_seg = None
            idx_t = None
            g0 = 0
            use_calls = calls[:max_calls] if "B" in phases else []
            for ci, (ch, bs, nb, o16) in enumerate(use_calls):
                if ci % cfg.IDXG == 0:
                    grp = use_calls[ci:ci + cfg.IDXG]
                    g0 = o16
                    gn = sum(c[2] for c in grp) * 8
                    idx_t = work.tile([128, cfg.GBLK * 8 * cfg.IDXG], I16, tag="idx")
                    nc.sync.dma_start(out=idx_t[:, :gn], in_=idx_dr[:, g0:g0 + gn])
                n16 = nb * 128 // 16
                gt = gathp.tile([128, cfg.GBLK, XP], F32, tag="g1")
                nc.gpsimd.dma_gather(
                    out_ap=gt[:, :nb, :], in_ap=x_full[ch * CH:(ch + 1) * CH, :],
                    idxs_ap=idx_t[:, o16 - g0:o16 - g0 + n16], num_idxs=nb * 128,
                    num_idxs_reg=nb * 128, elem_size=XP)
                for j in range(nb):
                    w, _ch, sf, sl = blocks[bs + j]
                    B = bs + j
                    wsize = min(128, NSH - w * 128)
                    sel = selp.tile([128, 128], F32, tag="sel")
                    nc.vector.tensor_scalar(
                        out=sel[:], in0=iota_f[:], scalar1=slot_sb[:, B:B + 1],
                        scalar2=val_sb[:, B:B + 1], op0=OP.is_equal, op1=OP.mult)
                    if sf:
                        ps_seg = psg.tile([128, 128], F32, tag="seg")
                    nc.tensor.matmul(out=ps_seg[:cfg.D_IN, :], lhsT=gt[:, j, :cfg.D_IN],
                                     rhs=sel[:], start=sf, stop=sl)
                    if sl:
                        nc.vector.tensor_add(
                            out=S1T[:, w * 128:w * 128 + wsize],
                            in0=S1T[:, w * 128:w * 128 + wsize],
                            in1=ps_seg[:cfg.D_IN, :wsize])

            if "B" in phases and "C" not in phases:
                # debug: dump S1T so phase B output is observable
                nc.sync.dma_start(
                    out=logits[:],
                    in_=x_full[cfg.NSH:2 * cfg.NSH, :cfg.NCLS])
            # ---------------- phase C: dense  h = relu(S1 W1 + b1); P = h W2 ----
            for d in (range(NW) if "C" in phases else []):
                wsize = min(128, NSH - d * 128)
                hts = []
                for j in range(NJ):
                    ps_h = ps4.tile([128, 128], F32, tag="ps")
                    nc.tensor.matmul(out=ps_h[:, :wsize],
                                     lhsT=W1_sb[:, j * 128:(j + 1) * 128],
                                     rhs=S1T[:, d * 128:d * 128 + wsize],
                                     start=True, stop=True)
                    ht = htp.tile([128, 128], F32, tag="ht")
                    nc.scalar.activation(out=ht[:, :wsize], in_=ps_h[:, :wsize],
                                         func=AF.Relu, bias=b1_sb[:, j:j + 1])
                    hts.append(ht)
                ps_p = psp.tile([128, PP], F32, tag="pps")
                for j in range(NJ):
                    nc.tensor.matmul(out=ps_p[:wsize], lhsT=hts[j][:, :wsize],
                                     rhs=W2_sb[:, j, :],
                                     start=(j == 0), stop=(j == NJ - 1))
                pt = work.tile([128, PP], F32, tag="pt")
                nc.vector.tensor_copy(out=pt[:wsize], in_=ps_p[:wsize])
                nc.sync.dma_start(out=p_sh[d * 128:d * 128 + wsize], in_=pt[:wsize])

            if "C" in phases:
                nc.gpsimd.collective_compute(
                    "AllGather", OP.bypass, replica_groups=groups,
                    ins=[p_sh[:]], outs=[p_full[:]])
                tc.strict_bb_all_engine_barrier()
            if "C" in phases and "D" not in phases:
                nc.sync.dma_start(out=logits[:],
                                  in_=p_full[:cfg.NSH, :cfg.NCLS])

            # ---------------- phase D: L2 SpMM  logits += A @ P ----------------
            ps_seg2 = None
            idx_t = None
            g0 = 0
            use_calls2 = calls if "D" in phases else []
            for ci, (ch, bs, nb, o16) in enumerate(use_calls2):
                if ci % cfg.IDXG == 0:
                    grp = use_calls2[ci:ci + cfg.IDXG]
                    g0 = o16
                    gn = sum(c[2] for c in grp) * 8
                    idx_t = work.tile([128, cfg.GBLK * 8 * cfg.IDXG], I16, tag="idx")
                    nc.sync.dma_start(out=idx_t[:, :gn], in_=idx_dr[:, g0:g0 + gn])
                n16 = nb * 128 // 16
                gt = gathp.tile([128, cfg.GBLK, PP], F32, tag="g2")
                nc.gpsimd.dma_gather(
                    out_ap=gt[:, :nb, :], in_ap=p_full[ch * CH:(ch + 1) * CH, :],
                    idxs_ap=idx_t[:, o16 - g0:o16 - g0 + n16], num_idxs=nb * 128,
                    num_idxs_reg=nb * 128, elem_size=PP)
                for j in range(nb):
                    w, _ch, sf, sl = blocks[bs + j]
                    B = bs + j
                    wsize = min(128, NSH - w * 128)
                    sel = selp.tile([128, 128], F32, tag="sel")
                    nc.vector.tensor_scalar(
                        out=sel[:], in0=iota_f[:], scalar1=slot_sb[:, B:B + 1],
                        scalar2=val_sb[:, B:B + 1], op0=OP.is_equal, op1=OP.mult)
                    if sf:
                        ps_seg2 = psg.tile([128, PP], F32, tag="seg")
                    nc.tensor.matmul(out=ps_seg2[:], lhsT=sel[:], rhs=gt[:, j, :],
                                     start=sf, stop=sl)
                    if sl:
                        nc.vector.tensor_add(
                            out=logit_sb[:wsize, w, :],
                            in0=logit_sb[:wsize, w, :],
                            in1=ps_seg2[:wsize])

            # ---------------- phase E: write logits ----------------
            if "D" not in phases:
                nfull = -1
            else:
                nfull = NSH // 128
            if nfull > 0:
                nc.sync.dma_start(
                    out=logits[:nfull * 128].rearrange("(d p) c -> p d c", p=128),
                    in_=logit_sb[:, :nfull, :cfg.NCLS])
            if nfull >= 0 and NSH % 128:
                tail = NSH % 128
                nc.sync.dma_start(out=logits[nfull * 128:],
                                  in_=logit_sb[:tail, nfull, :cfg.NCLS])

    nc.compile()
    return nc


# ----------------------------------------------------------------------------
# Entry point
# ----------------------------------------------------------------------------

def _run(cfg: Cfg, inputs: dict, trace: bool = False):
    in_maps, meta = build_host(cfg, inputs)
    nc = build_program(cfg, meta)
    res = run_bass_kernel_spmd(nc, in_maps, list(range(cfg.NCORES)), trace=trace)
    out = np.concatenate([res.results[k]["logits"] for k in range(cfg.NCORES)], axis=0)
    return out, res


def kernel(**inputs) -> np.ndarray:
    cfg = Cfg()
    out, _ = _run(cfg, inputs, trace=False)
    return out.astype(np.float32)


if __name__ == "__main__":
    # smoke test at reduced scale against a numpy reference
    cfg = Cfg(N=2048, E=32768, NCORES=8, NCHUNK=2)
    rng = np.random.default_rng(0)
    inputs = {
        "feature": rng.standard_normal((cfg.N, cfg.D_IN), dtype=np.float32),
        "conv_w": rng.standard_normal((4, 1, 5), dtype=np.float32) * 0.2,
        "conv_b": np.zeros(4, np.float32),
        "W1": rng.standard_normal((cfg.D_IN, cfg.D_HID), dtype=np.float32) * 0.1,
        "b1": np.zeros(cfg.D_HID, np.float32),
        "W2": rng.standard_normal((cfg.D_HID, cfg.NCLS), dtype=np.float32) * 0.05,
        "b2": np.zeros(cfg.NCLS, np.float32),
        "adj_val": rng.random(cfg.E, dtype=np.float32),
        "edge_row": rng.integers(0, cfg.N, cfg.E).astype(np.int32),
        "edge_col": rng.integers(0, cfg.N, cfg.E).astype(np.int32),
    }
    out, _ = _run(cfg, inputs)

    # numpy reference
    ws = inputs["conv_w"].sum(axis=0).ravel()
    xr = np.zeros((cfg.N, cfg.D_IN), np.float32)
    f = inputs["feature"]
    for k in range(5):
        s = k - 2
        lo, hi = max(0, -s), min(cfg.D_IN, cfg.D_IN - s)
        xr[:, lo:hi] += ws[k] * f[:, lo + s:hi + s]
    xr = np.maximum(xr + inputs["conv_b"].sum(), 0)
    S1 = np.zeros_like(xr)
    np.add.at(S1, inputs["edge_row"],
              inputs["adj_val"][:, None] * xr[inputs["edge_col"]])
    h = np.maximum(S1 @ inputs["W1"] + inputs["b1"], 0)
    P = h @ inputs["W2"]
    Y = np.zeros_like(P)
    np.add.at(Y, inputs["edge_row"], inputs["adj_val"][:, None] * P[inputs["edge_col"]])
    Y += inputs["b2"]
    err = np.abs(out - Y).max() / (np.abs(Y).max() + 1e-30)
    print("rel err:", err)

